# revision 24
# baseline (speedup 1.0000x reference)
"""NT-Xent / contrastive loss on 8 Trainium2 NeuronCores.

Reference computation (B=4096, D=512, temp=0.1):
    z   = l2norm(concat(proj_1, proj_2))          # [8192, 512]
    cos = (z @ z.T) / temp                        # [8192, 8192]
    pos[r]  = cos[r, (r + 4096) % 8192]
    lse[r]  = logsumexp(cos[r, :] with cos[r, r] masked out)
    loss    = mean(lse - pos)

Sharding: rows of the similarity matrix, 1024 per core.  Each core
receives the full stacked [8192, 512] input *rotated* by core*1024 rows,
which makes the program uniform across cores (SPMD): local rows 0..1023
are the core's rows, the self-diagonal sits at local column == row, and
the positive sits at local column == row + 4096.

Per core:
  1. SWDGE cast-DMA streams the 64 row-tiles in as bf16 (f32 read from
     HBM, bf16 write to SBUF) — no compute-engine cast pass.
  2. Row sumsq via one DVE scalar_tensor_tensor per tile (bf16 in);
     1/||row|| via fast-rsqrt (int magic) + 2 Newton steps on DVE.
  3. Normalization is folded into the PE transpose: instead of an
     identity, the transpose matmul's moving operand is diag(rn*S) so
     psT = z.T * rn * S drops out of the same 4 matmuls per tile.
     DVE evacuates PSUM straight to fp8e4 (S=64 keeps |zt| ~ 3).
  4. GEMM in fp8 with perf_mode=DoubleRow: contraction 512 done as two
     256-deep matmuls per 512-col chunk (2x PE throughput vs bf16).
     Columns grouped [1536,1536,1536,1536,1536,512] per row-block; one
     ScalarE Exp (scale=10/S^2) with accum_out per group gives the row
     sumexp.  Self/pos diagonals pulled out of raw PSUM with a
     multiply-by-identity reduce before the in-place Exp (self in
     group 0, pos in group 2 or 3, thanks to the input rotation).
  5. lse = ln(sumexp - exp(self*scale)); partial = sum(lse - scale*pos)
     over the core's 1024 rows, reduced to [1,1] via a ones-matmul.
Host adds the 8 partials and divides by 8192.

GEMM/transpose emission is interleaved so the Tile scheduler overlaps
the input stream (DMA/DVE/GpSimd) with the GEMM+exp pipeline (PE/ACT).
"""

import sys

import numpy as np

if "/opt/trn_rl_repo" not in sys.path:
    sys.path.insert(0, "/opt/trn_rl_repo")

_B = 4096
_D = 512
_N2 = 2 * _B            # 8192 rows of the similarity matrix
_NCORES = 8
_RPC = _N2 // _NCORES   # 1024 rows per core
_INV_TEMP = 10.0
_S = 64.0               # fp8 pre-scale on normalized rows
_SCL = _INV_TEMP / (_S * _S)   # logit scale applied at exp time

_NT = _N2 // 128        # 64 input row-tiles
_BATCHES = (8, 8, 16, 16, 16)   # tiles per load/rsqrt batch (fast start)
_NM = _RPC // 128       # 8 output row blocks per core
_NK = _D // 128         # 4 contraction chunks (2 DoubleRow pairs)
_NJ = _N2 // 512        # 16 column chunks of 512
# exp groups per row-block: small leading groups let the GEMM start after
# only 8 input tiles; 1536 amortizes ScalarE instruction overhead after.
_CGRP = (512, 1024, 1536, 1536, 1536, 1536, 512)

_MAGIC1 = 0x5F3759E0    # fast inverse sqrt magic + 1 (M - x == (M+1) + ~x)


def _emit(tc, projs, out_partial):
    import concourse.bass as bass  # noqa: F401
    from concourse import mybir

    nc = tc.nc
    f32 = mybir.dt.float32
    bf16 = mybir.dt.bfloat16
    fp8 = mybir.dt.float8e4
    i32 = mybir.dt.int32
    Alu = mybir.AluOpType
    Act = mybir.ActivationFunctionType
    DR = mybir.MatmulPerfMode.DoubleRow

    from contextlib import ExitStack
    ctx = ExitStack()
    pool = ctx.enter_context(tc.tile_pool(name="work", bufs=1))
    pers = ctx.enter_context(tc.tile_pool(name="pers", bufs=1))
    pspool = ctx.enter_context(tc.tile_pool(name="psum", bufs=1, space="PSUM"))

    # ---- constants ----
    ones = pers.tile([128, 128], f32, tag="ones")
    nc.vector.memset(ones[:], 1.0)
    ident = pers.tile([128, 128], f32, tag="ident")
    nc.gpsimd.affine_select(ident[:], ones[:], pattern=[[1, 128]],
                            compare_op=Alu.is_equal, fill=0.0,
                            base=0, channel_multiplier=-1)
    identb = pers.tile([128, 128], bf16, tag="identb")
    nc.vector.tensor_copy(identb[:], ident[:])

    # ---- persistent buffers ----
    # zT, normalized*S, fp8: K-chunk k lives at columns [k*8192, (k+1)*8192).
    zt = pers.tile([128, _NK * _N2], fp8, tag="zt")
    zt3 = zt.rearrange("p (k c) -> p k c", k=_NK)
    # whole rotated input, cast to bf16 by the DMA engines (SWDGE)
    raw_all = pers.tile([128, _NT * _D], bf16, tag="raw")
    raw3 = raw_all.rearrange("p (t d) -> p t d", t=_NT)
    sp_all = pers.tile([128, 2 * _NM], f32, tag="sp")    # self diag | pos diag
    rs_all = pers.tile([128, _NM], f32, tag="rs")        # row sumexp per block
    se_all = pers.tile([128, _NM * len(_CGRP)], f32, tag="se")  # group sums
    se3 = se_all.rearrange("p (m g) -> p m g", m=_NM)
    expd = pers.tile([128, max(_CGRP)], bf16, tag="expd")  # exp scratch out

    # ---- phase 1: issue every cast-DMA upfront (gpsimd queue only) ----
    # First two batches go per-tile so the first tiles land ASAP; the rest
    # are batched (fewer instructions, same serial SWDGE stream).
    def emit_all_dmas():
        for t in range(_NT):
            nc.gpsimd.dma_start(raw3[:, t, :],
                                projs[t * 128:(t + 1) * 128, :])

    # ---- per-batch compute chain (sumsq/rsqrt/diag/transpose/evac) ----
    def emit_load_group(g):
        t0 = sum(_BATCHES[:g])
        nb = _BATCHES[g]
        ssf = pool.tile([128, 16], f32, tag="ss", bufs=2, name=f"ss{g}")
        ss = ssf[:, 0:nb]
        for i in range(nb):
            t = t0 + i
            # row sumsq: split ScalarE (Square+accum) / DVE (STT) to
            # balance engine load; both land in ss[:, i].
            if t < 16:
                sq = pool.tile([128, _D], f32, tag="sqa", bufs=4,
                               name=f"sqa{t}")
                nc.scalar.activation(sq[:], raw3[:, t, :], Act.Square,
                                     bias=0.0, scale=1.0,
                                     accum_out=ss[:, i:i + 1])
            else:
                sq = pool.tile([128, _D], bf16, tag="sq", bufs=2,
                               name=f"sq{t}")
                nc.vector.scalar_tensor_tensor(
                    out=sq[:], in0=raw3[:, t, :], scalar=1.0,
                    in1=raw3[:, t, :],
                    op0=Alu.mult, op1=Alu.mult, accum_out=ss[:, i:i + 1])

        # rnorm = S/sqrt(max(ss, 1e-24)), fast-rsqrt + 2 Newton steps (DVE)
        sscf = pool.tile([128, 16], f32, tag="ssc", bufs=2, name=f"ssc{g}")
        ssc = sscf[:, 0:nb]
        nc.vector.tensor_scalar_max(ssc[:], ss[:], 1e-24)
        tif = pool.tile([128, 16], i32, tag="ti", bufs=2, name=f"ti{g}")
        ti = tif[:, 0:nb]
        nc.vector.tensor_scalar(
            out=ti[:], in0=ssc[:].bitcast(i32), scalar1=1, scalar2=-1,
            op0=Alu.logical_shift_right, op1=Alu.bitwise_xor)
        rnf = pool.tile([128, 16], f32, tag="rn", bufs=2, name=f"rn{g}")
        rn = rnf[:, 0:nb]
        nc.vector.tensor_scalar(
            out=rn[:].bitcast(i32), in0=ti[:], scalar1=_MAGIC1, scalar2=None,
            op0=Alu.add)
        ntf = pool.tile([128, 16], f32, tag="nt", bufs=2, name=f"nt{g}")
        nt = ntf[:, 0:nb]
        for _ in range(2):
            nc.vector.tensor_tensor(out=nt[:], in0=rn[:], in1=rn[:], op=Alu.mult)
            nc.vector.tensor_tensor(out=nt[:], in0=nt[:], in1=ssc[:], op=Alu.mult)
            nc.vector.tensor_scalar(out=nt[:], in0=nt[:], scalar1=-0.5,
                                    scalar2=1.5, op0=Alu.mult, op1=Alu.add)
            nc.vector.tensor_tensor(out=rn[:], in0=rn[:], in1=nt[:], op=Alu.mult)
        rnscf = pool.tile([128, 16], f32, tag="rnsc", bufs=2, name=f"rnsc{g}")
        rnsc = rnscf[:, 0:nb]
        nc.vector.tensor_scalar_mul(rnsc[:], rn[:], _S)

        for i in range(nb):
            t = t0 + i
            # diag(rn*S): identity column-scaled by per-partition scalar (DVE)
            diag = pool.tile([128, 128], bf16, tag="diag", bufs=8,
                             name=f"diag{t}")
            nc.vector.tensor_scalar_mul(diag[:], identb[:], rnsc[:, i:i + 1])
            # transpose + normalize in one: psT = raw.T @ diag(rn*S)
            psT = pspool.tile([128, _D], f32, tag="psT", bufs=2,
                              name=f"psT{t}")
            for d in range(_NK):
                nc.tensor.matmul(psT[:, d * 128:(d + 1) * 128],
                                 raw3[:, t, d * 128:(d + 1) * 128],
                                 diag[:], start=True, stop=True)
            # one strided evacuation: [128, 4, 128] f32 -> fp8
            dst = zt3[:, :, t * 128:(t + 1) * 128]
            src = psT[:].rearrange("p (k c) -> p k c", k=_NK)
            nc.vector.tensor_copy(dst, src)

    # ---- phase 2 helper: one (row-block m, col-group G) GEMM + exp ----
    def emit_gemm_group(m, G):
        width = _CGRP[G]
        col0 = sum(_CGRP[:G])
        psfull = pool_ps.tile([128, max(_CGRP)], f32, tag="ps", bufs=2,
                              name=f"ps{m}_{G}")
        ps = psfull[:, 0:width]
        # kk outer: consecutive matmuls share the stationary operand, so
        # LDWEIGHTS of the next chunk overlaps the running matmul cleanly.
        for kk in range(_NK // 2):
            for c in range(width // 512):
                j = col0 // 512 + c
                nc.tensor.matmul(
                    ps[:, c * 512:(c + 1) * 512],
                    zt3[:, 2 * kk:2 * kk + 2, m * 128:(m + 1) * 128],
                    zt3[:, 2 * kk:2 * kk + 2, j * 512:(j + 1) * 512],
                    start=(kk == 0), stop=(kk == _NK // 2 - 1),
                    perf_mode=DR)
        # diagonal extraction from raw PSUM (before in-place exp)
        selfoff = m * 128          # self diag lives in G0
        posoff = _B + m * 128      # pos diag in G2 (m<4) or G3 (m>=4)
        for col, off in ((m, selfoff), (_NM + m, posoff)):
            if col0 <= off and off + 128 <= col0 + width:
                junk = pool.tile([128, 128], f32, tag="junk", bufs=2,
                                 name=f"junk{m}_{G}")
                nc.vector.scalar_tensor_tensor(
                    out=junk[:], in0=ps[:, off - col0:off - col0 + 128],
                    scalar=1.0, in1=ident[:], op0=Alu.mult, op1=Alu.mult,
                    accum_out=sp_all[:, col:col + 1])
        nc.scalar.activation(expd[:, 0:width], ps[:], Act.Exp, bias=0.0,
                             scale=_SCL, accum_out=se3[:, m, G:G + 1])
        if G == len(_CGRP) - 1:
            nc.vector.reduce_sum(out=rs_all[:, m:m + 1], in_=se3[:, m, :],
                                 axis=mybir.AxisListType.X)

    pool_ps = pspool  # alias: GEMM psum groups live in the same pool

    # ---- interleaved emission: stream tiles, fire GEMM groups when fed ----
    # group G of row-block m needs zt columns up to col0+width, i.e. input
    # tiles < ceil((col0+width)/128); tiles arrive in load-group batches of 8.
    emit_all_dmas()
    next_g = 0

    def tiles_ready():
        return sum(_BATCHES[:next_g])

    for G in range(len(_CGRP)):
        need = (sum(_CGRP[:G + 1]) + 127) // 128
        need = max(need, 8)  # lhs panel: tiles 0..7
        while tiles_ready() < need:
            emit_load_group(next_g)
            next_g += 1
        for m in range(_NM):
            emit_gemm_group(m, G)
    while next_g < len(_BATCHES):
        emit_load_group(next_g)
        next_g += 1

    # ---- phase 3: lse, loss, partial sum ----
    sx = pool.tile([128, _NM], f32, tag="sx")
    nc.scalar.activation(sx[:], sp_all[:, 0:_NM], Act.Exp, bias=0.0,
                         scale=_SCL)
    nc.vector.tensor_sub(rs_all[:], rs_all[:], sx[:])
    lse = pool.tile([128, _NM], f32, tag="lse")
    nc.scalar.activation(lse[:], rs_all[:], Act.Ln, bias=0.0, scale=1.0)
    loss = pool.tile([128, _NM], f32, tag="loss")
    nc.vector.scalar_tensor_tensor(
        out=loss[:], in0=sp_all[:, _NM:2 * _NM], scalar=-_SCL,
        in1=lse[:], op0=Alu.mult, op1=Alu.add)
    lossv = pool.tile([128, 1], f32, tag="lossv")
    nc.vector.reduce_sum(out=lossv[:], in_=loss[:], axis=mybir.AxisListType.X)
    pf = pspool.tile([1, 1], f32, tag="psT", bufs=2)
    nc.tensor.matmul(pf[:], lossv[:], ones[:, 0:1], start=True, stop=True)
    res = pool.tile([1, 1], f32, tag="res")
    nc.vector.tensor_copy(res[:], pf[:])
    nc.sync.dma_start(out_partial[:, :], res[:])

    ctx.close()


def build():
    import concourse.tile as tile
    from concourse import bacc, mybir

    nc = bacc.Bacc("TRN2", target_bir_lowering=False, debug=False,
                   enable_asserts=True, num_devices=_NCORES)
    projs = nc.dram_tensor("projs", [_N2, _D], mybir.dt.float32,
                           kind="ExternalInput").ap()
    out_partial = nc.dram_tensor("partial", [1, 1], mybir.dt.float32,
                                 kind="ExternalOutput").ap()
    with tile.TileContext(nc) as tc:
        _emit(tc, projs, out_partial)
    nc.compile()
    return nc


_NC_CACHE = None


def _get_nc():
    global _NC_CACHE
    if _NC_CACHE is None:
        _NC_CACHE = build()
    return _NC_CACHE


def make_in_maps(proj_1, proj_2):
    z = np.concatenate([np.asarray(proj_1, dtype=np.float32),
                        np.asarray(proj_2, dtype=np.float32)], axis=0)
    return [{"projs": np.ascontiguousarray(np.roll(z, -_RPC * c, axis=0))}
            for c in range(_NCORES)]


def kernel(proj_1, proj_2):
    from concourse import bass_utils

    nc = _get_nc()
    in_maps = make_in_maps(proj_1, proj_2)
    r = bass_utils.run_bass_kernel_spmd(nc, in_maps,
                                        core_ids=list(range(_NCORES)))
    total = sum(float(res["partial"][0, 0]) for res in r.results)
    return np.float32(total / _N2)


# revision 25
# speedup vs baseline: 1.0198x; 1.0198x over previous
"""NT-Xent / contrastive loss on 8 Trainium2 NeuronCores.

Reference computation (B=4096, D=512, temp=0.1):
    z   = l2norm(concat(proj_1, proj_2))          # [8192, 512]
    cos = (z @ z.T) / temp                        # [8192, 8192]
    pos[r]  = cos[r, (r + 4096) % 8192]
    lse[r]  = logsumexp(cos[r, :] with cos[r, r] masked out)
    loss    = mean(lse - pos)

Sharding: rows of the similarity matrix, 1024 per core.  Each core
receives the full stacked [8192, 512] input *rotated* by core*1024 rows,
which makes the program uniform across cores (SPMD): local rows 0..1023
are the core's rows, the self-diagonal sits at local column == row, and
the positive sits at local column == row + 4096.

Per core:
  1. SWDGE cast-DMA streams the 64 row-tiles in as bf16 (f32 read from
     HBM, bf16 write to SBUF) — no compute-engine cast pass.
  2. Row sumsq via one DVE scalar_tensor_tensor per tile (bf16 in);
     1/||row|| via fast-rsqrt (int magic) + 2 Newton steps on DVE.
  3. Normalization is folded into the PE transpose: instead of an
     identity, the transpose matmul's moving operand is diag(rn*S) so
     psT = z.T * rn * S drops out of the same 4 matmuls per tile.
     DVE evacuates PSUM straight to fp8e4 (S=64 keeps |zt| ~ 3).
  4. GEMM in fp8 with perf_mode=DoubleRow: contraction 512 done as two
     256-deep matmuls per 512-col chunk (2x PE throughput vs bf16).
     Columns grouped [1536,1536,1536,1536,1536,512] per row-block; one
     ScalarE Exp (scale=10/S^2) with accum_out per group gives the row
     sumexp.  Self/pos diagonals pulled out of raw PSUM with a
     multiply-by-identity reduce before the in-place Exp (self in
     group 0, pos in group 2 or 3, thanks to the input rotation).
  5. lse = ln(sumexp - exp(self*scale)); partial = sum(lse - scale*pos)
     over the core's 1024 rows, reduced to [1,1] via a ones-matmul.
Host adds the 8 partials and divides by 8192.

GEMM/transpose emission is interleaved so the Tile scheduler overlaps
the input stream (DMA/DVE/GpSimd) with the GEMM+exp pipeline (PE/ACT).
"""

import sys

import numpy as np

if "/opt/trn_rl_repo" not in sys.path:
    sys.path.insert(0, "/opt/trn_rl_repo")

_B = 4096
_D = 512
_N2 = 2 * _B            # 8192 rows of the similarity matrix
_NCORES = 8
_RPC = _N2 // _NCORES   # 1024 rows per core
_INV_TEMP = 10.0
_S = 64.0               # fp8 pre-scale on normalized rows
_SCL = _INV_TEMP / (_S * _S)   # logit scale applied at exp time

_NT = _N2 // 128        # 64 input row-tiles
_BATCHES = (8, 8, 16, 16, 16)   # tiles per load/rsqrt batch (fast start)
_NM = _RPC // 128       # 8 output row blocks per core
_NK = _D // 128         # 4 contraction chunks (2 DoubleRow pairs)
_NJ = _N2 // 512        # 16 column chunks of 512
# exp groups per row-block: small leading groups let the GEMM start after
# only 8 input tiles; 1536 amortizes ScalarE instruction overhead after.
_CGRP = (512, 1024, 1536, 1536, 1536, 1536, 512)

_MAGIC1 = 0x5F3759E0    # fast inverse sqrt magic + 1 (M - x == (M+1) + ~x)


def _emit(tc, projs, out_partial):
    import concourse.bass as bass  # noqa: F401
    from concourse import mybir

    nc = tc.nc
    f32 = mybir.dt.float32
    bf16 = mybir.dt.bfloat16
    fp8 = mybir.dt.float8e4
    i32 = mybir.dt.int32
    Alu = mybir.AluOpType
    Act = mybir.ActivationFunctionType
    DR = mybir.MatmulPerfMode.DoubleRow

    from contextlib import ExitStack
    ctx = ExitStack()
    pool = ctx.enter_context(tc.tile_pool(name="work", bufs=1))
    pers = ctx.enter_context(tc.tile_pool(name="pers", bufs=1))
    pspool = ctx.enter_context(tc.tile_pool(name="psum", bufs=1, space="PSUM"))

    # ---- constants ----
    ones = pers.tile([128, 128], f32, tag="ones")
    nc.vector.memset(ones[:], 1.0)
    ident = pers.tile([128, 128], f32, tag="ident")
    nc.gpsimd.affine_select(ident[:], ones[:], pattern=[[1, 128]],
                            compare_op=Alu.is_equal, fill=0.0,
                            base=0, channel_multiplier=-1)
    identb = pers.tile([128, 128], bf16, tag="identb")
    nc.vector.tensor_copy(identb[:], ident[:])

    # ---- persistent buffers ----
    # zT, normalized*S, fp8: K-chunk k lives at columns [k*8192, (k+1)*8192).
    zt = pers.tile([128, _NK * _N2], fp8, tag="zt")
    zt3 = zt.rearrange("p (k c) -> p k c", k=_NK)
    # whole rotated input, cast to bf16 by the DMA engines (SWDGE)
    raw_all = pers.tile([128, _NT * _D], bf16, tag="raw")
    raw3 = raw_all.rearrange("p (t d) -> p t d", t=_NT)
    sp_all = pers.tile([128, 2 * _NM], f32, tag="sp")    # self diag | pos diag
    rs_all = pers.tile([128, _NM], f32, tag="rs")        # row sumexp per block
    se_all = pers.tile([128, _NM * len(_CGRP)], f32, tag="se")  # group sums
    se3 = se_all.rearrange("p (m g) -> p m g", m=_NM)
    expd = pers.tile([128, max(_CGRP)], bf16, tag="expd")  # exp scratch out

    # ---- phase 1: issue every cast-DMA upfront (gpsimd queue only) ----
    # First two batches go per-tile so the first tiles land ASAP; the rest
    # are batched (fewer instructions, same serial SWDGE stream).
    def emit_all_dmas():
        for t in range(_NT):
            nc.gpsimd.dma_start(raw3[:, t, :],
                                projs[t * 128:(t + 1) * 128, :])

    # ---- per-batch compute chain (sumsq/rsqrt/diag/transpose/evac) ----
    def emit_load_group(g):
        t0 = sum(_BATCHES[:g])
        nb = _BATCHES[g]
        ssf = pool.tile([128, 16], f32, tag="ss", bufs=2, name=f"ss{g}")
        ss = ssf[:, 0:nb]
        for i in range(nb):
            t = t0 + i
            # row sumsq: split ScalarE (Square+accum) / DVE (STT) to
            # balance engine load; both land in ss[:, i].
            if 8 <= t < 16:
                sq = pool.tile([128, _D], f32, tag="sqa", bufs=4,
                               name=f"sqa{t}")
                nc.scalar.activation(sq[:], raw3[:, t, :], Act.Square,
                                     bias=0.0, scale=1.0,
                                     accum_out=ss[:, i:i + 1])
            else:
                sq = pool.tile([128, _D], bf16, tag="sq", bufs=2,
                               name=f"sq{t}")
                nc.vector.scalar_tensor_tensor(
                    out=sq[:], in0=raw3[:, t, :], scalar=1.0,
                    in1=raw3[:, t, :],
                    op0=Alu.mult, op1=Alu.mult, accum_out=ss[:, i:i + 1])

        # rnorm = S/sqrt(max(ss, 1e-24)), fast-rsqrt + 2 Newton steps (DVE)
        sscf = pool.tile([128, 16], f32, tag="ssc", bufs=2, name=f"ssc{g}")
        ssc = sscf[:, 0:nb]
        nc.vector.tensor_scalar_max(ssc[:], ss[:], 1e-24)
        tif = pool.tile([128, 16], i32, tag="ti", bufs=2, name=f"ti{g}")
        ti = tif[:, 0:nb]
        nc.vector.tensor_scalar(
            out=ti[:], in0=ssc[:].bitcast(i32), scalar1=1, scalar2=-1,
            op0=Alu.logical_shift_right, op1=Alu.bitwise_xor)
        rnf = pool.tile([128, 16], f32, tag="rn", bufs=2, name=f"rn{g}")
        rn = rnf[:, 0:nb]
        nc.vector.tensor_scalar(
            out=rn[:].bitcast(i32), in0=ti[:], scalar1=_MAGIC1, scalar2=None,
            op0=Alu.add)
        ntf = pool.tile([128, 16], f32, tag="nt", bufs=2, name=f"nt{g}")
        nt = ntf[:, 0:nb]
        for _ in range(2):
            nc.vector.tensor_tensor(out=nt[:], in0=rn[:], in1=rn[:], op=Alu.mult)
            nc.vector.tensor_tensor(out=nt[:], in0=nt[:], in1=ssc[:], op=Alu.mult)
            nc.vector.tensor_scalar(out=nt[:], in0=nt[:], scalar1=-0.5,
                                    scalar2=1.5, op0=Alu.mult, op1=Alu.add)
            nc.vector.tensor_tensor(out=rn[:], in0=rn[:], in1=nt[:], op=Alu.mult)
        rnscf = pool.tile([128, 16], f32, tag="rnsc", bufs=2, name=f"rnsc{g}")
        rnsc = rnscf[:, 0:nb]
        nc.vector.tensor_scalar_mul(rnsc[:], rn[:], _S)

        for i in range(nb):
            t = t0 + i
            # diag(rn*S): identity column-scaled by per-partition scalar (DVE)
            diag = pool.tile([128, 128], bf16, tag="diag", bufs=8,
                             name=f"diag{t}")
            nc.vector.tensor_scalar_mul(diag[:], identb[:], rnsc[:, i:i + 1])
            # transpose + normalize in one: psT = raw.T @ diag(rn*S)
            psT = pspool.tile([128, _D], f32, tag="psT", bufs=2,
                              name=f"psT{t}")
            for d in range(_NK):
                nc.tensor.matmul(psT[:, d * 128:(d + 1) * 128],
                                 raw3[:, t, d * 128:(d + 1) * 128],
                                 diag[:], start=True, stop=True)
            # one strided evacuation: [128, 4, 128] f32 -> fp8
            dst = zt3[:, :, t * 128:(t + 1) * 128]
            src = psT[:].rearrange("p (k c) -> p k c", k=_NK)
            nc.vector.tensor_copy(dst, src)

    # ---- phase 2 helper: one (row-block m, col-group G) GEMM + exp ----
    def emit_gemm_group(m, G):
        width = _CGRP[G]
        col0 = sum(_CGRP[:G])
        psfull = pool_ps.tile([128, max(_CGRP)], f32, tag="ps", bufs=2,
                              name=f"ps{m}_{G}")
        ps = psfull[:, 0:width]
        # kk outer: consecutive matmuls share the stationary operand, so
        # LDWEIGHTS of the next chunk overlaps the running matmul cleanly.
        for kk in range(_NK // 2):
            for c in range(width // 512):
                j = col0 // 512 + c
                nc.tensor.matmul(
                    ps[:, c * 512:(c + 1) * 512],
                    zt3[:, 2 * kk:2 * kk + 2, m * 128:(m + 1) * 128],
                    zt3[:, 2 * kk:2 * kk + 2, j * 512:(j + 1) * 512],
                    start=(kk == 0), stop=(kk == _NK // 2 - 1),
                    perf_mode=DR)
        # diagonal extraction from raw PSUM (before in-place exp)
        selfoff = m * 128          # self diag lives in G0
        posoff = _B + m * 128      # pos diag in G2 (m<4) or G3 (m>=4)
        for col, off in ((m, selfoff), (_NM + m, posoff)):
            if col0 <= off and off + 128 <= col0 + width:
                junk = pool.tile([128, 128], f32, tag="junk", bufs=2,
                                 name=f"junk{m}_{G}")
                nc.vector.scalar_tensor_tensor(
                    out=junk[:], in0=ps[:, off - col0:off - col0 + 128],
                    scalar=1.0, in1=ident[:], op0=Alu.mult, op1=Alu.mult,
                    accum_out=sp_all[:, col:col + 1])
        nc.scalar.activation(expd[:, 0:width], ps[:], Act.Exp, bias=0.0,
                             scale=_SCL, accum_out=se3[:, m, G:G + 1])
        if G == len(_CGRP) - 1:
            nc.vector.reduce_sum(out=rs_all[:, m:m + 1], in_=se3[:, m, :],
                                 axis=mybir.AxisListType.X)

    pool_ps = pspool  # alias: GEMM psum groups live in the same pool

    # ---- interleaved emission: stream tiles, fire GEMM groups when fed ----
    # group G of row-block m needs zt columns up to col0+width, i.e. input
    # tiles < ceil((col0+width)/128); tiles arrive in load-group batches of 8.
    emit_all_dmas()
    next_g = 0

    def tiles_ready():
        return sum(_BATCHES[:next_g])

    for G in range(len(_CGRP)):
        need = (sum(_CGRP[:G + 1]) + 127) // 128
        need = max(need, 8)  # lhs panel: tiles 0..7
        while tiles_ready() < need:
            emit_load_group(next_g)
            next_g += 1
        for m in range(_NM):
            emit_gemm_group(m, G)
    while next_g < len(_BATCHES):
        emit_load_group(next_g)
        next_g += 1

    # ---- phase 3: lse, loss, partial sum ----
    sx = pool.tile([128, _NM], f32, tag="sx")
    nc.scalar.activation(sx[:], sp_all[:, 0:_NM], Act.Exp, bias=0.0,
                         scale=_SCL)
    nc.vector.tensor_sub(rs_all[:], rs_all[:], sx[:])
    lse = pool.tile([128, _NM], f32, tag="lse")
    nc.scalar.activation(lse[:], rs_all[:], Act.Ln, bias=0.0, scale=1.0)
    loss = pool.tile([128, _NM], f32, tag="loss")
    nc.vector.scalar_tensor_tensor(
        out=loss[:], in0=sp_all[:, _NM:2 * _NM], scalar=-_SCL,
        in1=lse[:], op0=Alu.mult, op1=Alu.add)
    lossv = pool.tile([128, 1], f32, tag="lossv")
    nc.vector.reduce_sum(out=lossv[:], in_=loss[:], axis=mybir.AxisListType.X)
    pf = pspool.tile([1, 1], f32, tag="psT", bufs=2)
    nc.tensor.matmul(pf[:], lossv[:], ones[:, 0:1], start=True, stop=True)
    res = pool.tile([1, 1], f32, tag="res")
    nc.vector.tensor_copy(res[:], pf[:])
    nc.sync.dma_start(out_partial[:, :], res[:])

    ctx.close()


def build():
    import concourse.tile as tile
    from concourse import bacc, mybir

    nc = bacc.Bacc("TRN2", target_bir_lowering=False, debug=False,
                   enable_asserts=True, num_devices=_NCORES)
    projs = nc.dram_tensor("projs", [_N2, _D], mybir.dt.float32,
                           kind="ExternalInput").ap()
    out_partial = nc.dram_tensor("partial", [1, 1], mybir.dt.float32,
                                 kind="ExternalOutput").ap()
    with tile.TileContext(nc) as tc:
        _emit(tc, projs, out_partial)
    nc.compile()
    return nc


_NC_CACHE = None


def _get_nc():
    global _NC_CACHE
    if _NC_CACHE is None:
        _NC_CACHE = build()
    return _NC_CACHE


def make_in_maps(proj_1, proj_2):
    z = np.concatenate([np.asarray(proj_1, dtype=np.float32),
                        np.asarray(proj_2, dtype=np.float32)], axis=0)
    return [{"projs": np.ascontiguousarray(np.roll(z, -_RPC * c, axis=0))}
            for c in range(_NCORES)]


def kernel(proj_1, proj_2):
    from concourse import bass_utils

    nc = _get_nc()
    in_maps = make_in_maps(proj_1, proj_2)
    r = bass_utils.run_bass_kernel_spmd(nc, in_maps,
                                        core_ids=list(range(_NCORES)))
    total = sum(float(res["partial"][0, 0]) for res in r.results)
    return np.float32(total / _N2)


# revision 26
# speedup vs baseline: 1.0372x; 1.0171x over previous
"""NT-Xent / contrastive loss on 8 Trainium2 NeuronCores.

Reference computation (B=4096, D=512, temp=0.1):
    z   = l2norm(concat(proj_1, proj_2))          # [8192, 512]
    cos = (z @ z.T) / temp                        # [8192, 8192]
    pos[r]  = cos[r, (r + 4096) % 8192]
    lse[r]  = logsumexp(cos[r, :] with cos[r, r] masked out)
    loss    = mean(lse - pos)

Sharding: rows of the similarity matrix, 1024 per core.  Each core
receives the full stacked [8192, 512] input *rotated* by core*1024 rows,
which makes the program uniform across cores (SPMD): local rows 0..1023
are the core's rows, the self-diagonal sits at local column == row, and
the positive sits at local column == row + 4096.

Per core:
  1. SWDGE cast-DMA streams the 64 row-tiles in as bf16 (f32 read from
     HBM, bf16 write to SBUF) — no compute-engine cast pass.
  2. Row sumsq via one DVE scalar_tensor_tensor per tile (bf16 in);
     1/||row|| via fast-rsqrt (int magic) + 2 Newton steps on DVE.
  3. Normalization is folded into the PE transpose: instead of an
     identity, the transpose matmul's moving operand is diag(rn*S) so
     psT = z.T * rn * S drops out of the same 4 matmuls per tile.
     DVE evacuates PSUM straight to fp8e4 (S=64 keeps |zt| ~ 3).
  4. GEMM in fp8 with perf_mode=DoubleRow: contraction 512 done as two
     256-deep matmuls per 512-col chunk (2x PE throughput vs bf16).
     Columns grouped [1536,1536,1536,1536,1536,512] per row-block; one
     ScalarE Exp (scale=10/S^2) with accum_out per group gives the row
     sumexp.  Self/pos diagonals pulled out of raw PSUM with a
     multiply-by-identity reduce before the in-place Exp (self in
     group 0, pos in group 2 or 3, thanks to the input rotation).
  5. lse = ln(sumexp - exp(self*scale)); partial = sum(lse - scale*pos)
     over the core's 1024 rows, reduced to [1,1] via a ones-matmul.
Host adds the 8 partials and divides by 8192.

GEMM/transpose emission is interleaved so the Tile scheduler overlaps
the input stream (DMA/DVE/GpSimd) with the GEMM+exp pipeline (PE/ACT).
"""

import sys

import numpy as np

if "/opt/trn_rl_repo" not in sys.path:
    sys.path.insert(0, "/opt/trn_rl_repo")

_B = 4096
_D = 512
_N2 = 2 * _B            # 8192 rows of the similarity matrix
_NCORES = 8
_RPC = _N2 // _NCORES   # 1024 rows per core
_INV_TEMP = 10.0
_S = 64.0               # fp8 pre-scale on normalized rows
_SCL = _INV_TEMP / (_S * _S)   # logit scale applied at exp time

_NT = _N2 // 128        # 64 input row-tiles
_BATCHES = (8, 8, 16, 16, 16)   # tiles per load/rsqrt batch (fast start)
_NM = _RPC // 128       # 8 output row blocks per core
_NK = _D // 128         # 4 contraction chunks (2 DoubleRow pairs)
_NJ = _N2 // 512        # 16 column chunks of 512
# exp groups per row-block: small leading groups let the GEMM start after
# only 8 input tiles; 1536 amortizes ScalarE instruction overhead after.
_CGRP = (512, 1024, 1536, 1536, 1536, 1536, 512)

_MAGIC1 = 0x5F3759E0    # fast inverse sqrt magic + 1 (M - x == (M+1) + ~x)


def _emit(tc, projs, out_partial):
    import concourse.bass as bass  # noqa: F401
    from concourse import mybir

    nc = tc.nc
    f32 = mybir.dt.float32
    bf16 = mybir.dt.bfloat16
    fp8 = mybir.dt.float8e4
    i32 = mybir.dt.int32
    Alu = mybir.AluOpType
    Act = mybir.ActivationFunctionType
    DR = mybir.MatmulPerfMode.DoubleRow

    from contextlib import ExitStack
    ctx = ExitStack()
    pool = ctx.enter_context(tc.tile_pool(name="work", bufs=1))
    pers = ctx.enter_context(tc.tile_pool(name="pers", bufs=1))
    pspool = ctx.enter_context(tc.tile_pool(name="psum", bufs=1, space="PSUM"))

    # ---- constants ----
    ones = pers.tile([128, 128], f32, tag="ones")
    nc.vector.memset(ones[:], 1.0)
    ident = pers.tile([128, 128], f32, tag="ident")
    nc.gpsimd.affine_select(ident[:], ones[:], pattern=[[1, 128]],
                            compare_op=Alu.is_equal, fill=0.0,
                            base=0, channel_multiplier=-1)
    identb = pers.tile([128, 128], bf16, tag="identb")
    nc.vector.tensor_copy(identb[:], ident[:])

    # ---- persistent buffers ----
    # zT, normalized*S, fp8: K-chunk k lives at columns [k*8192, (k+1)*8192).
    zt = pers.tile([128, _NK * _N2], fp8, tag="zt")
    zt3 = zt.rearrange("p (k c) -> p k c", k=_NK)
    # whole rotated input, cast to bf16 by the DMA engines (SWDGE)
    raw_all = pers.tile([128, _NT * _D], bf16, tag="raw")
    raw3 = raw_all.rearrange("p (t d) -> p t d", t=_NT)
    sp_all = pers.tile([128, 2 * _NM], f32, tag="sp")    # self diag | pos diag
    rs_all = pers.tile([128, _NM], f32, tag="rs")        # row sumexp per block
    se_all = pers.tile([128, _NM * len(_CGRP)], f32, tag="se")  # group sums
    se3 = se_all.rearrange("p (m g) -> p m g", m=_NM)
    expd = pers.tile([128, max(_CGRP)], bf16, tag="expd")  # exp scratch out

    # ---- phase 1: issue every cast-DMA upfront (gpsimd queue only) ----
    # First two batches go per-tile so the first tiles land ASAP; the rest
    # are batched (fewer instructions, same serial SWDGE stream).
    def emit_all_dmas():
        for t in range(_NT):
            nc.gpsimd.dma_start(raw3[:, t, :],
                                projs[t * 128:(t + 1) * 128, :])

    # ---- per-batch compute chain (sumsq/rsqrt/diag/transpose/evac) ----
    def emit_load_group(g):
        t0 = sum(_BATCHES[:g])
        nb = _BATCHES[g]
        ssf = pool.tile([128, 16], f32, tag="ss", bufs=2, name=f"ss{g}")
        ss = ssf[:, 0:nb]
        for i in range(nb):
            t = t0 + i
            # row sumsq: split ScalarE (Square+accum) / DVE (STT) to
            # balance engine load; both land in ss[:, i].
            if t < 16 or t % 16 in (0, 3, 6, 9, 12):
                sq = pool.tile([128, _D], f32, tag="sqa", bufs=4,
                               name=f"sqa{t}")
                nc.scalar.activation(sq[:], raw3[:, t, :], Act.Square,
                                     bias=0.0, scale=1.0,
                                     accum_out=ss[:, i:i + 1])
            else:
                sq = pool.tile([128, _D], bf16, tag="sq", bufs=2,
                               name=f"sq{t}")
                nc.vector.scalar_tensor_tensor(
                    out=sq[:], in0=raw3[:, t, :], scalar=1.0,
                    in1=raw3[:, t, :],
                    op0=Alu.mult, op1=Alu.mult, accum_out=ss[:, i:i + 1])

        # rnorm = S/sqrt(max(ss, 1e-24)), fast-rsqrt + 2 Newton steps (DVE)
        sscf = pool.tile([128, 16], f32, tag="ssc", bufs=2, name=f"ssc{g}")
        ssc = sscf[:, 0:nb]
        nc.vector.tensor_scalar_max(ssc[:], ss[:], 1e-24)
        tif = pool.tile([128, 16], i32, tag="ti", bufs=2, name=f"ti{g}")
        ti = tif[:, 0:nb]
        nc.vector.tensor_scalar(
            out=ti[:], in0=ssc[:].bitcast(i32), scalar1=1, scalar2=-1,
            op0=Alu.logical_shift_right, op1=Alu.bitwise_xor)
        rnf = pool.tile([128, 16], f32, tag="rn", bufs=2, name=f"rn{g}")
        rn = rnf[:, 0:nb]
        nc.vector.tensor_scalar(
            out=rn[:].bitcast(i32), in0=ti[:], scalar1=_MAGIC1, scalar2=None,
            op0=Alu.add)
        ntf = pool.tile([128, 16], f32, tag="nt", bufs=2, name=f"nt{g}")
        nt = ntf[:, 0:nb]
        for _ in range(2):
            nc.vector.tensor_tensor(out=nt[:], in0=rn[:], in1=rn[:], op=Alu.mult)
            nc.vector.tensor_tensor(out=nt[:], in0=nt[:], in1=ssc[:], op=Alu.mult)
            nc.vector.tensor_scalar(out=nt[:], in0=nt[:], scalar1=-0.5,
                                    scalar2=1.5, op0=Alu.mult, op1=Alu.add)
            nc.vector.tensor_tensor(out=rn[:], in0=rn[:], in1=nt[:], op=Alu.mult)
        rnscf = pool.tile([128, 16], f32, tag="rnsc", bufs=2, name=f"rnsc{g}")
        rnsc = rnscf[:, 0:nb]
        nc.vector.tensor_scalar_mul(rnsc[:], rn[:], _S)

        for i in range(nb):
            t = t0 + i
            # diag(rn*S): identity column-scaled by per-partition scalar (DVE)
            diag = pool.tile([128, 128], bf16, tag="diag", bufs=8,
                             name=f"diag{t}")
            nc.vector.tensor_scalar_mul(diag[:], identb[:], rnsc[:, i:i + 1])
            # transpose + normalize in one: psT = raw.T @ diag(rn*S)
            psT = pspool.tile([128, _D], f32, tag="psT", bufs=2,
                              name=f"psT{t}")
            for d in range(_NK):
                nc.tensor.matmul(psT[:, d * 128:(d + 1) * 128],
                                 raw3[:, t, d * 128:(d + 1) * 128],
                                 diag[:], start=True, stop=True)
            # one strided evacuation: [128, 4, 128] f32 -> fp8
            dst = zt3[:, :, t * 128:(t + 1) * 128]
            src = psT[:].rearrange("p (k c) -> p k c", k=_NK)
            nc.vector.tensor_copy(dst, src)

    # ---- phase 2 helper: one (row-block m, col-group G) GEMM + exp ----
    def emit_gemm_group(m, G):
        width = _CGRP[G]
        col0 = sum(_CGRP[:G])
        psfull = pool_ps.tile([128, max(_CGRP)], f32, tag="ps", bufs=2,
                              name=f"ps{m}_{G}")
        ps = psfull[:, 0:width]
        # kk outer: consecutive matmuls share the stationary operand, so
        # LDWEIGHTS of the next chunk overlaps the running matmul cleanly.
        for kk in range(_NK // 2):
            for c in range(width // 512):
                j = col0 // 512 + c
                nc.tensor.matmul(
                    ps[:, c * 512:(c + 1) * 512],
                    zt3[:, 2 * kk:2 * kk + 2, m * 128:(m + 1) * 128],
                    zt3[:, 2 * kk:2 * kk + 2, j * 512:(j + 1) * 512],
                    start=(kk == 0), stop=(kk == _NK // 2 - 1),
                    perf_mode=DR)
        # diagonal extraction from raw PSUM (before in-place exp)
        selfoff = m * 128          # self diag lives in G0
        posoff = _B + m * 128      # pos diag in G2 (m<4) or G3 (m>=4)
        for col, off in ((m, selfoff), (_NM + m, posoff)):
            if col0 <= off and off + 128 <= col0 + width:
                junk = pool.tile([128, 128], f32, tag="junk", bufs=2,
                                 name=f"junk{m}_{G}")
                nc.vector.scalar_tensor_tensor(
                    out=junk[:], in0=ps[:, off - col0:off - col0 + 128],
                    scalar=1.0, in1=ident[:], op0=Alu.mult, op1=Alu.mult,
                    accum_out=sp_all[:, col:col + 1])
        nc.scalar.activation(expd[:, 0:width], ps[:], Act.Exp, bias=0.0,
                             scale=_SCL, accum_out=se3[:, m, G:G + 1])
        if G == len(_CGRP) - 1:
            nc.vector.reduce_sum(out=rs_all[:, m:m + 1], in_=se3[:, m, :],
                                 axis=mybir.AxisListType.X)

    pool_ps = pspool  # alias: GEMM psum groups live in the same pool

    # ---- interleaved emission: stream tiles, fire GEMM groups when fed ----
    # group G of row-block m needs zt columns up to col0+width, i.e. input
    # tiles < ceil((col0+width)/128); tiles arrive in load-group batches of 8.
    emit_all_dmas()
    next_g = 0

    def tiles_ready():
        return sum(_BATCHES[:next_g])

    for G in range(len(_CGRP)):
        need = (sum(_CGRP[:G + 1]) + 127) // 128
        need = max(need, 8)  # lhs panel: tiles 0..7
        while tiles_ready() < need:
            emit_load_group(next_g)
            next_g += 1
        for m in range(_NM):
            emit_gemm_group(m, G)
    while next_g < len(_BATCHES):
        emit_load_group(next_g)
        next_g += 1

    # ---- phase 3: lse, loss, partial sum ----
    sx = pool.tile([128, _NM], f32, tag="sx")
    nc.scalar.activation(sx[:], sp_all[:, 0:_NM], Act.Exp, bias=0.0,
                         scale=_SCL)
    nc.vector.tensor_sub(rs_all[:], rs_all[:], sx[:])
    lse = pool.tile([128, _NM], f32, tag="lse")
    nc.scalar.activation(lse[:], rs_all[:], Act.Ln, bias=0.0, scale=1.0)
    loss = pool.tile([128, _NM], f32, tag="loss")
    nc.vector.scalar_tensor_tensor(
        out=loss[:], in0=sp_all[:, _NM:2 * _NM], scalar=-_SCL,
        in1=lse[:], op0=Alu.mult, op1=Alu.add)
    lossv = pool.tile([128, 1], f32, tag="lossv")
    nc.vector.reduce_sum(out=lossv[:], in_=loss[:], axis=mybir.AxisListType.X)
    pf = pspool.tile([1, 1], f32, tag="psT", bufs=2)
    nc.tensor.matmul(pf[:], lossv[:], ones[:, 0:1], start=True, stop=True)
    res = pool.tile([1, 1], f32, tag="res")
    nc.vector.tensor_copy(res[:], pf[:])
    nc.sync.dma_start(out_partial[:, :], res[:])

    ctx.close()


def build():
    import concourse.tile as tile
    from concourse import bacc, mybir

    nc = bacc.Bacc("TRN2", target_bir_lowering=False, debug=False,
                   enable_asserts=True, num_devices=_NCORES)
    projs = nc.dram_tensor("projs", [_N2, _D], mybir.dt.float32,
                           kind="ExternalInput").ap()
    out_partial = nc.dram_tensor("partial", [1, 1], mybir.dt.float32,
                                 kind="ExternalOutput").ap()
    with tile.TileContext(nc) as tc:
        _emit(tc, projs, out_partial)
    nc.compile()
    return nc


_NC_CACHE = None


def _get_nc():
    global _NC_CACHE
    if _NC_CACHE is None:
        _NC_CACHE = build()
    return _NC_CACHE


def make_in_maps(proj_1, proj_2):
    z = np.concatenate([np.asarray(proj_1, dtype=np.float32),
                        np.asarray(proj_2, dtype=np.float32)], axis=0)
    return [{"projs": np.ascontiguousarray(np.roll(z, -_RPC * c, axis=0))}
            for c in range(_NCORES)]


def kernel(proj_1, proj_2):
    from concourse import bass_utils

    nc = _get_nc()
    in_maps = make_in_maps(proj_1, proj_2)
    r = bass_utils.run_bass_kernel_spmd(nc, in_maps,
                                        core_ids=list(range(_NCORES)))
    total = sum(float(res["partial"][0, 0]) for res in r.results)
    return np.float32(total / _N2)


# revision 27
# speedup vs baseline: 1.0833x; 1.0444x over previous
"""NT-Xent / contrastive loss on 8 Trainium2 NeuronCores.

Reference computation (B=4096, D=512, temp=0.1):
    z   = l2norm(concat(proj_1, proj_2))          # [8192, 512]
    cos = (z @ z.T) / temp                        # [8192, 8192]
    pos[r]  = cos[r, (r + 4096) % 8192]
    lse[r]  = logsumexp(cos[r, :] with cos[r, r] masked out)
    loss    = mean(lse - pos)

Sharding: rows of the similarity matrix, 1024 per core.  Each core
receives the full stacked [8192, 512] input *rotated* by core*1024 rows,
which makes the program uniform across cores (SPMD): local rows 0..1023
are the core's rows, the self-diagonal sits at local column == row, and
the positive sits at local column == row + 4096.

Per core:
  1. SWDGE cast-DMA streams the 64 row-tiles in as bf16 (f32 read from
     HBM, bf16 write to SBUF) — no compute-engine cast pass.
  2. Row sumsq via one DVE scalar_tensor_tensor per tile (bf16 in);
     1/||row|| via fast-rsqrt (int magic) + 2 Newton steps on DVE.
  3. Normalization is folded into the PE transpose: instead of an
     identity, the transpose matmul's moving operand is diag(rn*S) so
     psT = z.T * rn * S drops out of the same 4 matmuls per tile.
     DVE evacuates PSUM straight to fp8e4 (S=64 keeps |zt| ~ 3).
  4. GEMM in fp8 with perf_mode=DoubleRow: contraction 512 done as two
     256-deep matmuls per 512-col chunk (2x PE throughput vs bf16).
     Columns grouped [1536,1536,1536,1536,1536,512] per row-block; one
     ScalarE Exp (scale=10/S^2) with accum_out per group gives the row
     sumexp.  Self/pos diagonals pulled out of raw PSUM with a
     multiply-by-identity reduce before the in-place Exp (self in
     group 0, pos in group 2 or 3, thanks to the input rotation).
  5. lse = ln(sumexp - exp(self*scale)); partial = sum(lse - scale*pos)
     over the core's 1024 rows, reduced to [1,1] via a ones-matmul.
Host adds the 8 partials and divides by 8192.

GEMM/transpose emission is interleaved so the Tile scheduler overlaps
the input stream (DMA/DVE/GpSimd) with the GEMM+exp pipeline (PE/ACT).
"""

import sys

import numpy as np

if "/opt/trn_rl_repo" not in sys.path:
    sys.path.insert(0, "/opt/trn_rl_repo")

_B = 4096
_D = 512
_N2 = 2 * _B            # 8192 rows of the similarity matrix
_NCORES = 8
_RPC = _N2 // _NCORES   # 1024 rows per core
_INV_TEMP = 10.0
_S = 64.0               # fp8 pre-scale on normalized rows
_SCL = _INV_TEMP / (_S * _S)   # logit scale applied at exp time

_NT = _N2 // 128        # 64 input row-tiles
_BATCHES = (8, 8, 16, 16, 16)   # tiles per load/rsqrt batch (fast start)
_NM = _RPC // 128       # 8 output row blocks per core
_NK = _D // 128         # 4 contraction chunks (2 DoubleRow pairs)
_NJ = _N2 // 512        # 16 column chunks of 512
# exp groups per row-block: small leading groups let the GEMM start after
# only 8 input tiles; 1536 amortizes ScalarE instruction overhead after.
_CGRP = (512, 1024, 1536, 1536, 1536, 1536, 512)

_MAGIC1 = 0x5F3759E0    # fast inverse sqrt magic + 1 (M - x == (M+1) + ~x)


def _emit(tc, projs, out_partial):
    import concourse.bass as bass  # noqa: F401
    from concourse import mybir

    nc = tc.nc
    f32 = mybir.dt.float32
    bf16 = mybir.dt.bfloat16
    fp8 = mybir.dt.float8e4
    i32 = mybir.dt.int32
    Alu = mybir.AluOpType
    Act = mybir.ActivationFunctionType
    DR = mybir.MatmulPerfMode.DoubleRow

    from contextlib import ExitStack
    ctx = ExitStack()
    pool = ctx.enter_context(tc.tile_pool(name="work", bufs=1))
    pers = ctx.enter_context(tc.tile_pool(name="pers", bufs=1))
    pspool = ctx.enter_context(tc.tile_pool(name="psum", bufs=1, space="PSUM"))

    # ---- constants ----
    ones = pers.tile([128, 128], f32, tag="ones")
    nc.vector.memset(ones[:], 1.0)
    ident = pers.tile([128, 128], f32, tag="ident")
    nc.gpsimd.affine_select(ident[:], ones[:], pattern=[[1, 128]],
                            compare_op=Alu.is_equal, fill=0.0,
                            base=0, channel_multiplier=-1)
    identb = pers.tile([128, 128], bf16, tag="identb")
    nc.vector.tensor_copy(identb[:], ident[:])

    # ---- persistent buffers ----
    # zT, normalized*S, fp8: K-chunk k lives at columns [k*8192, (k+1)*8192).
    zt = pers.tile([128, _NK * _N2], fp8, tag="zt")
    zt3 = zt.rearrange("p (k c) -> p k c", k=_NK)
    # whole rotated input, cast to bf16 by the DMA engines (SWDGE)
    raw_all = pers.tile([128, _NT * _D], bf16, tag="raw")
    raw3 = raw_all.rearrange("p (t d) -> p t d", t=_NT)
    sp_all = pers.tile([128, 2 * _NM], f32, tag="sp")    # self diag | pos diag
    rs_all = pers.tile([128, _NM], f32, tag="rs")        # row sumexp per block
    se_all = pers.tile([128, _NM * len(_CGRP)], f32, tag="se")  # group sums
    se3 = se_all.rearrange("p (m g) -> p m g", m=_NM)

    # ---- phase 1: issue every cast-DMA upfront (gpsimd queue only) ----
    # First two batches go per-tile so the first tiles land ASAP; the rest
    # are batched (fewer instructions, same serial SWDGE stream).
    def emit_all_dmas():
        for t in range(_NT):
            nc.gpsimd.dma_start(raw3[:, t, :],
                                projs[t * 128:(t + 1) * 128, :])

    # ---- per-batch compute chain (sumsq/rsqrt/diag/transpose/evac) ----
    def emit_load_group(g):
        t0 = sum(_BATCHES[:g])
        nb = _BATCHES[g]
        ssf = pool.tile([128, 16], f32, tag="ss", bufs=2, name=f"ss{g}")
        ss = ssf[:, 0:nb]
        for i in range(nb):
            t = t0 + i
            # row sumsq: split ScalarE (Square+accum) / DVE (STT) to
            # balance engine load; both land in ss[:, i].
            if t < 16 or t % 16 in (0, 3, 6, 9, 12):
                sq = pool.tile([128, _D], f32, tag="sqa", bufs=4,
                               name=f"sqa{t}")
                nc.scalar.activation(sq[:], raw3[:, t, :], Act.Square,
                                     bias=0.0, scale=1.0,
                                     accum_out=ss[:, i:i + 1])
            else:
                sq = pool.tile([128, _D], bf16, tag="sq", bufs=2,
                               name=f"sq{t}")
                nc.vector.scalar_tensor_tensor(
                    out=sq[:], in0=raw3[:, t, :], scalar=1.0,
                    in1=raw3[:, t, :],
                    op0=Alu.mult, op1=Alu.mult, accum_out=ss[:, i:i + 1])

        # rnorm = S/sqrt(max(ss, 1e-24)), fast-rsqrt + 2 Newton steps (DVE)
        sscf = pool.tile([128, 16], f32, tag="ssc", bufs=2, name=f"ssc{g}")
        ssc = sscf[:, 0:nb]
        nc.vector.tensor_scalar_max(ssc[:], ss[:], 1e-24)
        tif = pool.tile([128, 16], i32, tag="ti", bufs=2, name=f"ti{g}")
        ti = tif[:, 0:nb]
        nc.vector.tensor_scalar(
            out=ti[:], in0=ssc[:].bitcast(i32), scalar1=1, scalar2=-1,
            op0=Alu.logical_shift_right, op1=Alu.bitwise_xor)
        rnf = pool.tile([128, 16], f32, tag="rn", bufs=2, name=f"rn{g}")
        rn = rnf[:, 0:nb]
        nc.vector.tensor_scalar(
            out=rn[:].bitcast(i32), in0=ti[:], scalar1=_MAGIC1, scalar2=None,
            op0=Alu.add)
        ntf = pool.tile([128, 16], f32, tag="nt", bufs=2, name=f"nt{g}")
        nt = ntf[:, 0:nb]
        for _ in range(2):
            nc.vector.tensor_tensor(out=nt[:], in0=rn[:], in1=rn[:], op=Alu.mult)
            nc.vector.tensor_tensor(out=nt[:], in0=nt[:], in1=ssc[:], op=Alu.mult)
            nc.vector.tensor_scalar(out=nt[:], in0=nt[:], scalar1=-0.5,
                                    scalar2=1.5, op0=Alu.mult, op1=Alu.add)
            nc.vector.tensor_tensor(out=rn[:], in0=rn[:], in1=nt[:], op=Alu.mult)
        rnscf = pool.tile([128, 16], f32, tag="rnsc", bufs=2, name=f"rnsc{g}")
        rnsc = rnscf[:, 0:nb]
        nc.vector.tensor_scalar_mul(rnsc[:], rn[:], _S)

        for i in range(nb):
            t = t0 + i
            # diag(rn*S): identity column-scaled by per-partition scalar (DVE)
            diag = pool.tile([128, 128], bf16, tag="diag", bufs=8,
                             name=f"diag{t}")
            nc.vector.tensor_scalar_mul(diag[:], identb[:], rnsc[:, i:i + 1])
            # transpose + normalize in one: psT = raw.T @ diag(rn*S)
            psT = pspool.tile([128, _D], f32, tag="psT", bufs=2,
                              name=f"psT{t}")
            for d in range(_NK):
                nc.tensor.matmul(psT[:, d * 128:(d + 1) * 128],
                                 raw3[:, t, d * 128:(d + 1) * 128],
                                 diag[:], start=True, stop=True)
            # one strided evacuation: [128, 4, 128] f32 -> fp8
            dst = zt3[:, :, t * 128:(t + 1) * 128]
            src = psT[:].rearrange("p (k c) -> p k c", k=_NK)
            nc.vector.tensor_copy(dst, src)

    # ---- phase 2 helper: one (row-block m, col-group G) GEMM + exp ----
    def emit_gemm_group(m, G):
        width = _CGRP[G]
        col0 = sum(_CGRP[:G])
        psfull = pool_ps.tile([128, max(_CGRP)], f32, tag="ps", bufs=2,
                              name=f"ps{m}_{G}")
        ps = psfull[:, 0:width]
        # kk outer: consecutive matmuls share the stationary operand, so
        # LDWEIGHTS of the next chunk overlaps the running matmul cleanly.
        for kk in range(_NK // 2):
            for c in range(width // 512):
                j = col0 // 512 + c
                nc.tensor.matmul(
                    ps[:, c * 512:(c + 1) * 512],
                    zt3[:, 2 * kk:2 * kk + 2, m * 128:(m + 1) * 128],
                    zt3[:, 2 * kk:2 * kk + 2, j * 512:(j + 1) * 512],
                    start=(kk == 0), stop=(kk == _NK // 2 - 1),
                    perf_mode=DR)
        # diagonal extraction from raw PSUM (before in-place exp)
        selfoff = m * 128          # self diag lives in G0
        posoff = _B + m * 128      # pos diag in G2 (m<4) or G3 (m>=4)
        for col, off in ((m, selfoff), (_NM + m, posoff)):
            if col0 <= off and off + 128 <= col0 + width:
                junk = pool.tile([128, 128], f32, tag="junk", bufs=2,
                                 name=f"junk{m}_{G}")
                nc.vector.scalar_tensor_tensor(
                    out=junk[:], in0=ps[:, off - col0:off - col0 + 128],
                    scalar=1.0, in1=ident[:], op0=Alu.mult, op1=Alu.mult,
                    accum_out=sp_all[:, col:col + 1])
        nc.scalar.activation(ps[:], ps[:], Act.Exp, bias=0.0,
                             scale=_SCL, accum_out=se3[:, m, G:G + 1])
        if G == len(_CGRP) - 1:
            nc.vector.reduce_sum(out=rs_all[:, m:m + 1], in_=se3[:, m, :],
                                 axis=mybir.AxisListType.X)

    pool_ps = pspool  # alias: GEMM psum groups live in the same pool

    # ---- interleaved emission: stream tiles, fire GEMM groups when fed ----
    # group G of row-block m needs zt columns up to col0+width, i.e. input
    # tiles < ceil((col0+width)/128); tiles arrive in load-group batches of 8.
    emit_all_dmas()
    next_g = 0

    def tiles_ready():
        return sum(_BATCHES[:next_g])

    for G in range(len(_CGRP)):
        need = (sum(_CGRP[:G + 1]) + 127) // 128
        need = max(need, 8)  # lhs panel: tiles 0..7
        while tiles_ready() < need:
            emit_load_group(next_g)
            next_g += 1
        for m in range(_NM):
            emit_gemm_group(m, G)
    while next_g < len(_BATCHES):
        emit_load_group(next_g)
        next_g += 1

    # ---- phase 3: lse, loss, partial sum ----
    sx = pool.tile([128, _NM], f32, tag="sx")
    nc.scalar.activation(sx[:], sp_all[:, 0:_NM], Act.Exp, bias=0.0,
                         scale=_SCL)
    nc.vector.tensor_sub(rs_all[:], rs_all[:], sx[:])
    lse = pool.tile([128, _NM], f32, tag="lse")
    nc.scalar.activation(lse[:], rs_all[:], Act.Ln, bias=0.0, scale=1.0)
    loss = pool.tile([128, _NM], f32, tag="loss")
    nc.vector.scalar_tensor_tensor(
        out=loss[:], in0=sp_all[:, _NM:2 * _NM], scalar=-_SCL,
        in1=lse[:], op0=Alu.mult, op1=Alu.add)
    lossv = pool.tile([128, 1], f32, tag="lossv")
    nc.vector.reduce_sum(out=lossv[:], in_=loss[:], axis=mybir.AxisListType.X)
    pf = pspool.tile([1, 1], f32, tag="psT", bufs=2)
    nc.tensor.matmul(pf[:], lossv[:], ones[:, 0:1], start=True, stop=True)
    res = pool.tile([1, 1], f32, tag="res")
    nc.vector.tensor_copy(res[:], pf[:])
    nc.sync.dma_start(out_partial[:, :], res[:])

    ctx.close()


def build():
    import concourse.tile as tile
    from concourse import bacc, mybir

    nc = bacc.Bacc("TRN2", target_bir_lowering=False, debug=False,
                   enable_asserts=True, num_devices=_NCORES)
    projs = nc.dram_tensor("projs", [_N2, _D], mybir.dt.float32,
                           kind="ExternalInput").ap()
    out_partial = nc.dram_tensor("partial", [1, 1], mybir.dt.float32,
                                 kind="ExternalOutput").ap()
    with tile.TileContext(nc) as tc:
        _emit(tc, projs, out_partial)
    nc.compile()
    return nc


_NC_CACHE = None


def _get_nc():
    global _NC_CACHE
    if _NC_CACHE is None:
        _NC_CACHE = build()
    return _NC_CACHE


def make_in_maps(proj_1, proj_2):
    z = np.concatenate([np.asarray(proj_1, dtype=np.float32),
                        np.asarray(proj_2, dtype=np.float32)], axis=0)
    return [{"projs": np.ascontiguousarray(np.roll(z, -_RPC * c, axis=0))}
            for c in range(_NCORES)]


def kernel(proj_1, proj_2):
    from concourse import bass_utils

    nc = _get_nc()
    in_maps = make_in_maps(proj_1, proj_2)
    r = bass_utils.run_bass_kernel_spmd(nc, in_maps,
                                        core_ids=list(range(_NCORES)))
    total = sum(float(res["partial"][0, 0]) for res in r.results)
    return np.float32(total / _N2)


# revision 28
# speedup vs baseline: 1.1530x; 1.0644x over previous
"""NT-Xent / contrastive loss on 8 Trainium2 NeuronCores.

Reference computation (B=4096, D=512, temp=0.1):
    z   = l2norm(concat(proj_1, proj_2))          # [8192, 512]
    cos = (z @ z.T) / temp                        # [8192, 8192]
    pos[r]  = cos[r, (r + 4096) % 8192]
    lse[r]  = logsumexp(cos[r, :] with cos[r, r] masked out)
    loss    = mean(lse - pos)

Sharding: rows of the similarity matrix, 1024 per core.  Each core
receives the full stacked [8192, 512] input *rotated* by core*1024 rows,
which makes the program uniform across cores (SPMD): local rows 0..1023
are the core's rows, the self-diagonal sits at local column == row, and
the positive sits at local column == row + 4096.

Per core:
  1. SWDGE cast-DMA streams the 64 row-tiles in as bf16 (f32 read from
     HBM, bf16 write to SBUF) — no compute-engine cast pass.
  2. Row sumsq via one DVE scalar_tensor_tensor per tile (bf16 in);
     1/||row|| via fast-rsqrt (int magic) + 2 Newton steps on DVE.
  3. Normalization is folded into the PE transpose: instead of an
     identity, the transpose matmul's moving operand is diag(rn*S) so
     psT = z.T * rn * S drops out of the same 4 matmuls per tile.
     DVE evacuates PSUM straight to fp8e4 (S=64 keeps |zt| ~ 3).
  4. GEMM in fp8 with perf_mode=DoubleRow: contraction 512 done as two
     256-deep matmuls per 512-col chunk (2x PE throughput vs bf16).
     Columns grouped [1536,1536,1536,1536,1536,512] per row-block; one
     ScalarE Exp (scale=10/S^2) with accum_out per group gives the row
     sumexp.  Self/pos diagonals pulled out of raw PSUM with a
     multiply-by-identity reduce before the in-place Exp (self in
     group 0, pos in group 2 or 3, thanks to the input rotation).
  5. lse = ln(sumexp - exp(self*scale)); partial = sum(lse - scale*pos)
     over the core's 1024 rows, reduced to [1,1] via a ones-matmul.
Host adds the 8 partials and divides by 8192.

GEMM/transpose emission is interleaved so the Tile scheduler overlaps
the input stream (DMA/DVE/GpSimd) with the GEMM+exp pipeline (PE/ACT).
"""

import sys

import numpy as np

if "/opt/trn_rl_repo" not in sys.path:
    sys.path.insert(0, "/opt/trn_rl_repo")

_B = 4096
_D = 512
_N2 = 2 * _B            # 8192 rows of the similarity matrix
_NCORES = 8
_RPC = _N2 // _NCORES   # 1024 rows per core
_INV_TEMP = 10.0
_S = 64.0               # fp8 pre-scale on normalized rows
_SCL = _INV_TEMP / (_S * _S)   # logit scale applied at exp time

_NT = _N2 // 128        # 64 input row-tiles
_BATCHES = (8, 8, 8, 8, 8, 8, 8, 8)   # tiles per load/rsqrt batch
_NM = _RPC // 128       # 8 output row blocks per core
_NK = _D // 128         # 4 contraction chunks (2 DoubleRow pairs)
_NJ = _N2 // 512        # 16 column chunks of 512
# exp groups per row-block: small leading groups let the GEMM start after
# only 8 input tiles; 1536 amortizes ScalarE instruction overhead after.
_CGRP = (512, 1024, 1536, 1536, 1536, 1536, 512)

_MAGIC1 = 0x5F3759E0    # fast inverse sqrt magic + 1 (M - x == (M+1) + ~x)


def _emit(tc, projs, out_partial):
    import concourse.bass as bass  # noqa: F401
    from concourse import mybir

    nc = tc.nc
    f32 = mybir.dt.float32
    bf16 = mybir.dt.bfloat16
    fp8 = mybir.dt.float8e4
    i32 = mybir.dt.int32
    Alu = mybir.AluOpType
    Act = mybir.ActivationFunctionType
    DR = mybir.MatmulPerfMode.DoubleRow

    from contextlib import ExitStack
    ctx = ExitStack()
    pool = ctx.enter_context(tc.tile_pool(name="work", bufs=1))
    pers = ctx.enter_context(tc.tile_pool(name="pers", bufs=1))
    pspool = ctx.enter_context(tc.tile_pool(name="psum", bufs=1, space="PSUM"))

    # ---- constants ----
    ones = pers.tile([128, 128], f32, tag="ones")
    nc.vector.memset(ones[:], 1.0)
    ident = pers.tile([128, 128], f32, tag="ident")
    nc.gpsimd.affine_select(ident[:], ones[:], pattern=[[1, 128]],
                            compare_op=Alu.is_equal, fill=0.0,
                            base=0, channel_multiplier=-1)
    identb = pers.tile([128, 128], bf16, tag="identb")
    nc.vector.tensor_copy(identb[:], ident[:])

    # ---- persistent buffers ----
    # zT, normalized*S, fp8: K-chunk k lives at columns [k*8192, (k+1)*8192).
    zt = pers.tile([128, _NK * _N2], fp8, tag="zt")
    zt3 = zt.rearrange("p (k c) -> p k c", k=_NK)
    # whole rotated input, cast to bf16 by the DMA engines (SWDGE)
    raw_all = pers.tile([128, _NT * _D], bf16, tag="raw")
    raw3 = raw_all.rearrange("p (t d) -> p t d", t=_NT)
    sp_all = pers.tile([128, 2 * _NM], f32, tag="sp")    # self diag | pos diag
    rs_all = pers.tile([128, _NM], f32, tag="rs")        # row sumexp per block
    se_all = pers.tile([128, _NM * len(_CGRP)], f32, tag="se")  # group sums
    se3 = se_all.rearrange("p (m g) -> p m g", m=_NM)

    # ---- phase 1: issue every cast-DMA upfront (gpsimd queue only) ----
    # First two batches go per-tile so the first tiles land ASAP; the rest
    # are batched (fewer instructions, same serial SWDGE stream).
    def emit_all_dmas():
        for t in range(_NT):
            nc.gpsimd.dma_start(raw3[:, t, :],
                                projs[t * 128:(t + 1) * 128, :])

    # ---- per-batch compute chain (sumsq/rsqrt/diag/transpose/evac) ----
    def emit_load_group(g):
        t0 = sum(_BATCHES[:g])
        nb = _BATCHES[g]
        ssf = pool.tile([128, 16], f32, tag="ss", bufs=2, name=f"ss{g}")
        ss = ssf[:, 0:nb]
        for i in range(nb):
            t = t0 + i
            # row sumsq: split ScalarE (Square+accum) / DVE (STT) to
            # balance engine load; both land in ss[:, i].
            if t < 16 or t % 16 in (0, 3, 6, 9, 12):
                sq = pool.tile([128, _D], f32, tag="sqa", bufs=4,
                               name=f"sqa{t}")
                nc.scalar.activation(sq[:], raw3[:, t, :], Act.Square,
                                     bias=0.0, scale=1.0,
                                     accum_out=ss[:, i:i + 1])
            else:
                sq = pool.tile([128, _D], bf16, tag="sq", bufs=2,
                               name=f"sq{t}")
                nc.vector.scalar_tensor_tensor(
                    out=sq[:], in0=raw3[:, t, :], scalar=1.0,
                    in1=raw3[:, t, :],
                    op0=Alu.mult, op1=Alu.mult, accum_out=ss[:, i:i + 1])

        # rnorm = S/sqrt(max(ss, 1e-24)), fast-rsqrt + 2 Newton steps (DVE)
        sscf = pool.tile([128, 16], f32, tag="ssc", bufs=2, name=f"ssc{g}")
        ssc = sscf[:, 0:nb]
        nc.vector.tensor_scalar_max(ssc[:], ss[:], 1e-24)
        tif = pool.tile([128, 16], i32, tag="ti", bufs=2, name=f"ti{g}")
        ti = tif[:, 0:nb]
        nc.vector.tensor_scalar(
            out=ti[:], in0=ssc[:].bitcast(i32), scalar1=1, scalar2=-1,
            op0=Alu.logical_shift_right, op1=Alu.bitwise_xor)
        rnf = pool.tile([128, 16], f32, tag="rn", bufs=2, name=f"rn{g}")
        rn = rnf[:, 0:nb]
        nc.vector.tensor_scalar(
            out=rn[:].bitcast(i32), in0=ti[:], scalar1=_MAGIC1, scalar2=None,
            op0=Alu.add)
        ntf = pool.tile([128, 16], f32, tag="nt", bufs=2, name=f"nt{g}")
        nt = ntf[:, 0:nb]
        for _ in range(2):
            nc.vector.tensor_tensor(out=nt[:], in0=rn[:], in1=rn[:], op=Alu.mult)
            nc.vector.tensor_tensor(out=nt[:], in0=nt[:], in1=ssc[:], op=Alu.mult)
            nc.vector.tensor_scalar(out=nt[:], in0=nt[:], scalar1=-0.5,
                                    scalar2=1.5, op0=Alu.mult, op1=Alu.add)
            nc.vector.tensor_tensor(out=rn[:], in0=rn[:], in1=nt[:], op=Alu.mult)
        rnscf = pool.tile([128, 16], f32, tag="rnsc", bufs=2, name=f"rnsc{g}")
        rnsc = rnscf[:, 0:nb]
        nc.vector.tensor_scalar_mul(rnsc[:], rn[:], _S)

        for i in range(nb):
            t = t0 + i
            # diag(rn*S): identity column-scaled by per-partition scalar (DVE)
            diag = pool.tile([128, 128], bf16, tag="diag", bufs=8,
                             name=f"diag{t}")
            nc.vector.tensor_scalar_mul(diag[:], identb[:], rnsc[:, i:i + 1])
            # transpose + normalize in one: psT = raw.T @ diag(rn*S)
            psT = pspool.tile([128, _D], f32, tag="psT", bufs=2,
                              name=f"psT{t}")
            for d in range(_NK):
                nc.tensor.matmul(psT[:, d * 128:(d + 1) * 128],
                                 raw3[:, t, d * 128:(d + 1) * 128],
                                 diag[:], start=True, stop=True)
            # one strided evacuation: [128, 4, 128] f32 -> fp8
            dst = zt3[:, :, t * 128:(t + 1) * 128]
            src = psT[:].rearrange("p (k c) -> p k c", k=_NK)
            nc.vector.tensor_copy(dst, src)

    # ---- phase 2 helper: one (row-block m, col-group G) GEMM + exp ----
    def emit_gemm_group(m, G):
        width = _CGRP[G]
        col0 = sum(_CGRP[:G])
        psfull = pool_ps.tile([128, max(_CGRP)], f32, tag="ps", bufs=2,
                              name=f"ps{m}_{G}")
        ps = psfull[:, 0:width]
        # kk outer: consecutive matmuls share the stationary operand, so
        # LDWEIGHTS of the next chunk overlaps the running matmul cleanly.
        for kk in range(_NK // 2):
            for c in range(width // 512):
                j = col0 // 512 + c
                nc.tensor.matmul(
                    ps[:, c * 512:(c + 1) * 512],
                    zt3[:, 2 * kk:2 * kk + 2, m * 128:(m + 1) * 128],
                    zt3[:, 2 * kk:2 * kk + 2, j * 512:(j + 1) * 512],
                    start=(kk == 0), stop=(kk == _NK // 2 - 1),
                    perf_mode=DR)
        # diagonal extraction from raw PSUM (before in-place exp)
        selfoff = m * 128          # self diag lives in G0
        posoff = _B + m * 128      # pos diag in G2 (m<4) or G3 (m>=4)
        for col, off in ((m, selfoff), (_NM + m, posoff)):
            if col0 <= off and off + 128 <= col0 + width:
                junk = pool.tile([128, 128], f32, tag="junk", bufs=2,
                                 name=f"junk{m}_{G}")
                nc.vector.scalar_tensor_tensor(
                    out=junk[:], in0=ps[:, off - col0:off - col0 + 128],
                    scalar=1.0, in1=ident[:], op0=Alu.mult, op1=Alu.mult,
                    accum_out=sp_all[:, col:col + 1])
        nc.scalar.activation(ps[:], ps[:], Act.Exp, bias=0.0,
                             scale=_SCL, accum_out=se3[:, m, G:G + 1])
        if G == len(_CGRP) - 1:
            nc.vector.reduce_sum(out=rs_all[:, m:m + 1], in_=se3[:, m, :],
                                 axis=mybir.AxisListType.X)

    pool_ps = pspool  # alias: GEMM psum groups live in the same pool

    # ---- interleaved emission: stream tiles, fire GEMM groups when fed ----
    # group G of row-block m needs zt columns up to col0+width, i.e. input
    # tiles < ceil((col0+width)/128); tiles arrive in load-group batches of 8.
    emit_all_dmas()
    next_g = 0

    def tiles_ready():
        return sum(_BATCHES[:next_g])

    for G in range(len(_CGRP)):
        need = (sum(_CGRP[:G + 1]) + 127) // 128
        need = max(need, 8)  # lhs panel: tiles 0..7
        while tiles_ready() < need:
            emit_load_group(next_g)
            next_g += 1
        for m in range(_NM):
            emit_gemm_group(m, G)
    while next_g < len(_BATCHES):
        emit_load_group(next_g)
        next_g += 1

    # ---- phase 3: lse, loss, partial sum ----
    sx = pool.tile([128, _NM], f32, tag="sx")
    nc.scalar.activation(sx[:], sp_all[:, 0:_NM], Act.Exp, bias=0.0,
                         scale=_SCL)
    nc.vector.tensor_sub(rs_all[:], rs_all[:], sx[:])
    lse = pool.tile([128, _NM], f32, tag="lse")
    nc.scalar.activation(lse[:], rs_all[:], Act.Ln, bias=0.0, scale=1.0)
    loss = pool.tile([128, _NM], f32, tag="loss")
    nc.vector.scalar_tensor_tensor(
        out=loss[:], in0=sp_all[:, _NM:2 * _NM], scalar=-_SCL,
        in1=lse[:], op0=Alu.mult, op1=Alu.add)
    lossv = pool.tile([128, 1], f32, tag="lossv")
    nc.vector.reduce_sum(out=lossv[:], in_=loss[:], axis=mybir.AxisListType.X)
    pf = pspool.tile([1, 1], f32, tag="psT", bufs=2)
    nc.tensor.matmul(pf[:], lossv[:], ones[:, 0:1], start=True, stop=True)
    res = pool.tile([1, 1], f32, tag="res")
    nc.vector.tensor_copy(res[:], pf[:])
    nc.sync.dma_start(out_partial[:, :], res[:])

    ctx.close()


def build():
    import concourse.tile as tile
    from concourse import bacc, mybir

    nc = bacc.Bacc("TRN2", target_bir_lowering=False, debug=False,
                   enable_asserts=True, num_devices=_NCORES)
    projs = nc.dram_tensor("projs", [_N2, _D], mybir.dt.float32,
                           kind="ExternalInput").ap()
    out_partial = nc.dram_tensor("partial", [1, 1], mybir.dt.float32,
                                 kind="ExternalOutput").ap()
    with tile.TileContext(nc) as tc:
        _emit(tc, projs, out_partial)
    nc.compile()
    return nc


_NC_CACHE = None


def _get_nc():
    global _NC_CACHE
    if _NC_CACHE is None:
        _NC_CACHE = build()
    return _NC_CACHE


def make_in_maps(proj_1, proj_2):
    z = np.concatenate([np.asarray(proj_1, dtype=np.float32),
                        np.asarray(proj_2, dtype=np.float32)], axis=0)
    return [{"projs": np.ascontiguousarray(np.roll(z, -_RPC * c, axis=0))}
            for c in range(_NCORES)]


def kernel(proj_1, proj_2):
    from concourse import bass_utils

    nc = _get_nc()
    in_maps = make_in_maps(proj_1, proj_2)
    r = bass_utils.run_bass_kernel_spmd(nc, in_maps,
                                        core_ids=list(range(_NCORES)))
    total = sum(float(res["partial"][0, 0]) for res in r.results)
    return np.float32(total / _N2)


# revision 29
# speedup vs baseline: 1.2252x; 1.0626x over previous
"""NT-Xent / contrastive loss on 8 Trainium2 NeuronCores.

Reference computation (B=4096, D=512, temp=0.1):
    z   = l2norm(concat(proj_1, proj_2))          # [8192, 512]
    cos = (z @ z.T) / temp                        # [8192, 8192]
    pos[r]  = cos[r, (r + 4096) % 8192]
    lse[r]  = logsumexp(cos[r, :] with cos[r, r] masked out)
    loss    = mean(lse - pos)

Sharding: rows of the similarity matrix, 1024 per core.  Each core
receives the full stacked [8192, 512] input *rotated* by core*1024 rows,
which makes the program uniform across cores (SPMD): local rows 0..1023
are the core's rows, the self-diagonal sits at local column == row, and
the positive sits at local column == row + 4096.

Per core:
  1. SWDGE cast-DMA streams the 64 row-tiles in as bf16 (f32 read from
     HBM, bf16 write to SBUF) — no compute-engine cast pass.
  2. Row sumsq via one DVE scalar_tensor_tensor per tile (bf16 in);
     1/||row|| via fast-rsqrt (int magic) + 2 Newton steps on DVE.
  3. Normalization is folded into the PE transpose: instead of an
     identity, the transpose matmul's moving operand is diag(rn*S) so
     psT = z.T * rn * S drops out of the same 4 matmuls per tile.
     DVE evacuates PSUM straight to fp8e4 (S=64 keeps |zt| ~ 3).
  4. GEMM in fp8 with perf_mode=DoubleRow: contraction 512 done as two
     256-deep matmuls per 512-col chunk (2x PE throughput vs bf16).
     Columns grouped [1536,1536,1536,1536,1536,512] per row-block; one
     ScalarE Exp (scale=10/S^2) with accum_out per group gives the row
     sumexp.  Self/pos diagonals pulled out of raw PSUM with a
     multiply-by-identity reduce before the in-place Exp (self in
     group 0, pos in group 2 or 3, thanks to the input rotation).
  5. lse = ln(sumexp - exp(self*scale)); partial = sum(lse - scale*pos)
     over the core's 1024 rows, reduced to [1,1] via a ones-matmul.
Host adds the 8 partials and divides by 8192.

GEMM/transpose emission is interleaved so the Tile scheduler overlaps
the input stream (DMA/DVE/GpSimd) with the GEMM+exp pipeline (PE/ACT).
"""

import sys

import numpy as np

if "/opt/trn_rl_repo" not in sys.path:
    sys.path.insert(0, "/opt/trn_rl_repo")

_B = 4096
_D = 512
_N2 = 2 * _B            # 8192 rows of the similarity matrix
_NCORES = 8
_RPC = _N2 // _NCORES   # 1024 rows per core
_INV_TEMP = 10.0
_S = 64.0               # fp8 pre-scale on normalized rows
_SCL = _INV_TEMP / (_S * _S)   # logit scale applied at exp time

_NT = _N2 // 128        # 64 input row-tiles
_BATCHES = (8, 8, 8, 8, 8, 8, 8, 8)   # tiles per load/rsqrt batch
_NM = _RPC // 128       # 8 output row blocks per core
_NK = _D // 128         # 4 contraction chunks (2 DoubleRow pairs)
_NJ = _N2 // 512        # 16 column chunks of 512
# exp groups per row-block: small leading groups let the GEMM start after
# only 8 input tiles; 1536 amortizes ScalarE instruction overhead after.
_CGRP = (512, 1024, 1536, 1536, 1536, 1536, 512)

_MAGIC1 = 0x5F3759E0    # fast inverse sqrt magic + 1 (M - x == (M+1) + ~x)


def _emit(tc, projs, out_partial):
    import concourse.bass as bass  # noqa: F401
    from concourse import mybir

    nc = tc.nc
    f32 = mybir.dt.float32
    bf16 = mybir.dt.bfloat16
    fp8 = mybir.dt.float8e4
    i32 = mybir.dt.int32
    Alu = mybir.AluOpType
    Act = mybir.ActivationFunctionType
    DR = mybir.MatmulPerfMode.DoubleRow

    from contextlib import ExitStack
    ctx = ExitStack()
    pool = ctx.enter_context(tc.tile_pool(name="work", bufs=1))
    pers = ctx.enter_context(tc.tile_pool(name="pers", bufs=1))
    pspool = ctx.enter_context(tc.tile_pool(name="psum", bufs=1, space="PSUM"))

    # ---- constants ----
    ones = pers.tile([128, 128], f32, tag="ones")
    nc.vector.memset(ones[:], 1.0)
    ident = pers.tile([128, 128], f32, tag="ident")
    nc.gpsimd.affine_select(ident[:], ones[:], pattern=[[1, 128]],
                            compare_op=Alu.is_equal, fill=0.0,
                            base=0, channel_multiplier=-1)
    identb = pers.tile([128, 128], bf16, tag="identb")
    nc.vector.tensor_copy(identb[:], ident[:])

    # ---- persistent buffers ----
    # zT, normalized*S, fp8: K-chunk k lives at columns [k*8192, (k+1)*8192).
    zt = pers.tile([128, _NK * _N2], fp8, tag="zt")
    zt3 = zt.rearrange("p (k c) -> p k c", k=_NK)
    # whole rotated input, cast to bf16 by the DMA engines (SWDGE)
    raw_all = pers.tile([128, _NT * _D], bf16, tag="raw")
    raw3 = raw_all.rearrange("p (t d) -> p t d", t=_NT)
    sp_all = pers.tile([128, 2 * _NM], f32, tag="sp")    # self diag | pos diag
    rs_all = pers.tile([128, _NM], f32, tag="rs")        # row sumexp per block
    se_all = pers.tile([128, _NM * len(_CGRP)], f32, tag="se")  # group sums
    se3 = se_all.rearrange("p (m g) -> p m g", m=_NM)

    # ---- phase 1: issue every cast-DMA upfront (gpsimd queue only) ----
    # First two batches go per-tile so the first tiles land ASAP; the rest
    # are batched (fewer instructions, same serial SWDGE stream).
    def emit_all_dmas():
        for t in range(_NT):
            nc.gpsimd.dma_start(raw3[:, t, :],
                                projs[t * 128:(t + 1) * 128, :])

    # ---- per-batch compute chain (sumsq/rsqrt/diag/transpose/evac) ----
    def emit_load_group(g):
        t0 = sum(_BATCHES[:g])
        nb = _BATCHES[g]
        ssf = pool.tile([128, 16], f32, tag="ss", bufs=2, name=f"ss{g}")
        ss = ssf[:, 0:nb]
        for i in range(nb):
            t = t0 + i
            # row sumsq: split ScalarE (Square+accum) / DVE (STT) to
            # balance engine load; both land in ss[:, i].
            if 8 <= t < 16 or (t >= 16 and t % 16 in (0, 3, 6, 9, 12)):
                sq = pool.tile([128, _D], f32, tag="sqa", bufs=4,
                               name=f"sqa{t}")
                nc.scalar.activation(sq[:], raw3[:, t, :], Act.Square,
                                     bias=0.0, scale=1.0,
                                     accum_out=ss[:, i:i + 1])
            else:
                sq = pool.tile([128, _D], bf16, tag="sq", bufs=2,
                               name=f"sq{t}")
                nc.vector.scalar_tensor_tensor(
                    out=sq[:], in0=raw3[:, t, :], scalar=1.0,
                    in1=raw3[:, t, :],
                    op0=Alu.mult, op1=Alu.mult, accum_out=ss[:, i:i + 1])

        # rnorm = S/sqrt(max(ss, 1e-24)), fast-rsqrt + 2 Newton steps (DVE)
        sscf = pool.tile([128, 16], f32, tag="ssc", bufs=2, name=f"ssc{g}")
        ssc = sscf[:, 0:nb]
        nc.vector.tensor_scalar_max(ssc[:], ss[:], 1e-24)
        tif = pool.tile([128, 16], i32, tag="ti", bufs=2, name=f"ti{g}")
        ti = tif[:, 0:nb]
        nc.vector.tensor_scalar(
            out=ti[:], in0=ssc[:].bitcast(i32), scalar1=1, scalar2=-1,
            op0=Alu.logical_shift_right, op1=Alu.bitwise_xor)
        rnf = pool.tile([128, 16], f32, tag="rn", bufs=2, name=f"rn{g}")
        rn = rnf[:, 0:nb]
        nc.vector.tensor_scalar(
            out=rn[:].bitcast(i32), in0=ti[:], scalar1=_MAGIC1, scalar2=None,
            op0=Alu.add)
        ntf = pool.tile([128, 16], f32, tag="nt", bufs=2, name=f"nt{g}")
        nt = ntf[:, 0:nb]
        for _ in range(2):
            nc.vector.tensor_tensor(out=nt[:], in0=rn[:], in1=rn[:], op=Alu.mult)
            nc.vector.tensor_tensor(out=nt[:], in0=nt[:], in1=ssc[:], op=Alu.mult)
            nc.vector.tensor_scalar(out=nt[:], in0=nt[:], scalar1=-0.5,
                                    scalar2=1.5, op0=Alu.mult, op1=Alu.add)
            nc.vector.tensor_tensor(out=rn[:], in0=rn[:], in1=nt[:], op=Alu.mult)
        rnscf = pool.tile([128, 16], f32, tag="rnsc", bufs=2, name=f"rnsc{g}")
        rnsc = rnscf[:, 0:nb]
        nc.vector.tensor_scalar_mul(rnsc[:], rn[:], _S)

        for i in range(nb):
            t = t0 + i
            # diag(rn*S): identity column-scaled by per-partition scalar (DVE)
            diag = pool.tile([128, 128], bf16, tag="diag", bufs=8,
                             name=f"diag{t}")
            nc.vector.tensor_scalar_mul(diag[:], identb[:], rnsc[:, i:i + 1])
            # transpose + normalize in one: psT = raw.T @ diag(rn*S)
            psT = pspool.tile([128, _D], f32, tag="psT", bufs=2,
                              name=f"psT{t}")
            for d in range(_NK):
                nc.tensor.matmul(psT[:, d * 128:(d + 1) * 128],
                                 raw3[:, t, d * 128:(d + 1) * 128],
                                 diag[:], start=True, stop=True)
            # one strided evacuation: [128, 4, 128] f32 -> fp8
            dst = zt3[:, :, t * 128:(t + 1) * 128]
            src = psT[:].rearrange("p (k c) -> p k c", k=_NK)
            nc.vector.tensor_copy(dst, src)

    # ---- phase 2 helper: one (row-block m, col-group G) GEMM + exp ----
    def emit_gemm_group(m, G):
        width = _CGRP[G]
        col0 = sum(_CGRP[:G])
        psfull = pool_ps.tile([128, max(_CGRP)], f32, tag="ps", bufs=2,
                              name=f"ps{m}_{G}")
        ps = psfull[:, 0:width]
        # kk outer: consecutive matmuls share the stationary operand, so
        # LDWEIGHTS of the next chunk overlaps the running matmul cleanly.
        for kk in range(_NK // 2):
            for c in range(width // 512):
                j = col0 // 512 + c
                nc.tensor.matmul(
                    ps[:, c * 512:(c + 1) * 512],
                    zt3[:, 2 * kk:2 * kk + 2, m * 128:(m + 1) * 128],
                    zt3[:, 2 * kk:2 * kk + 2, j * 512:(j + 1) * 512],
                    start=(kk == 0), stop=(kk == _NK // 2 - 1),
                    perf_mode=DR)
        # diagonal extraction from raw PSUM (before in-place exp)
        selfoff = m * 128          # self diag lives in G0
        posoff = _B + m * 128      # pos diag in G2 (m<4) or G3 (m>=4)
        for col, off in ((m, selfoff), (_NM + m, posoff)):
            if col0 <= off and off + 128 <= col0 + width:
                junk = pool.tile([128, 128], f32, tag="junk", bufs=2,
                                 name=f"junk{m}_{G}")
                nc.vector.scalar_tensor_tensor(
                    out=junk[:], in0=ps[:, off - col0:off - col0 + 128],
                    scalar=1.0, in1=ident[:], op0=Alu.mult, op1=Alu.mult,
                    accum_out=sp_all[:, col:col + 1])
        nc.scalar.activation(ps[:], ps[:], Act.Exp, bias=0.0,
                             scale=_SCL, accum_out=se3[:, m, G:G + 1])
        if G == len(_CGRP) - 1:
            nc.vector.reduce_sum(out=rs_all[:, m:m + 1], in_=se3[:, m, :],
                                 axis=mybir.AxisListType.X)

    pool_ps = pspool  # alias: GEMM psum groups live in the same pool

    # ---- interleaved emission: stream tiles, fire GEMM groups when fed ----
    # group G of row-block m needs zt columns up to col0+width, i.e. input
    # tiles < ceil((col0+width)/128); tiles arrive in load-group batches of 8.
    emit_all_dmas()
    next_g = 0

    def tiles_ready():
        return sum(_BATCHES[:next_g])

    for G in range(len(_CGRP)):
        need = (sum(_CGRP[:G + 1]) + 127) // 128
        need = max(need, 8)  # lhs panel: tiles 0..7
        while tiles_ready() < need:
            emit_load_group(next_g)
            next_g += 1
        for m in range(_NM):
            emit_gemm_group(m, G)
    while next_g < len(_BATCHES):
        emit_load_group(next_g)
        next_g += 1

    # ---- phase 3: lse, loss, partial sum ----
    sx = pool.tile([128, _NM], f32, tag="sx")
    nc.scalar.activation(sx[:], sp_all[:, 0:_NM], Act.Exp, bias=0.0,
                         scale=_SCL)
    nc.vector.tensor_sub(rs_all[:], rs_all[:], sx[:])
    lse = pool.tile([128, _NM], f32, tag="lse")
    nc.scalar.activation(lse[:], rs_all[:], Act.Ln, bias=0.0, scale=1.0)
    loss = pool.tile([128, _NM], f32, tag="loss")
    nc.vector.scalar_tensor_tensor(
        out=loss[:], in0=sp_all[:, _NM:2 * _NM], scalar=-_SCL,
        in1=lse[:], op0=Alu.mult, op1=Alu.add)
    lossv = pool.tile([128, 1], f32, tag="lossv")
    nc.vector.reduce_sum(out=lossv[:], in_=loss[:], axis=mybir.AxisListType.X)
    pf = pspool.tile([1, 1], f32, tag="psT", bufs=2)
    nc.tensor.matmul(pf[:], lossv[:], ones[:, 0:1], start=True, stop=True)
    res = pool.tile([1, 1], f32, tag="res")
    nc.vector.tensor_copy(res[:], pf[:])
    nc.sync.dma_start(out_partial[:, :], res[:])

    ctx.close()


def build():
    import concourse.tile as tile
    from concourse import bacc, mybir

    nc = bacc.Bacc("TRN2", target_bir_lowering=False, debug=False,
                   enable_asserts=True, num_devices=_NCORES)
    projs = nc.dram_tensor("projs", [_N2, _D], mybir.dt.float32,
                           kind="ExternalInput").ap()
    out_partial = nc.dram_tensor("partial", [1, 1], mybir.dt.float32,
                                 kind="ExternalOutput").ap()
    with tile.TileContext(nc) as tc:
        _emit(tc, projs, out_partial)
    nc.compile()
    return nc


_NC_CACHE = None


def _get_nc():
    global _NC_CACHE
    if _NC_CACHE is None:
        _NC_CACHE = build()
    return _NC_CACHE


def make_in_maps(proj_1, proj_2):
    z = np.concatenate([np.asarray(proj_1, dtype=np.float32),
                        np.asarray(proj_2, dtype=np.float32)], axis=0)
    return [{"projs": np.ascontiguousarray(np.roll(z, -_RPC * c, axis=0))}
            for c in range(_NCORES)]


def kernel(proj_1, proj_2):
    from concourse import bass_utils

    nc = _get_nc()
    in_maps = make_in_maps(proj_1, proj_2)
    r = bass_utils.run_bass_kernel_spmd(nc, in_maps,
                                        core_ids=list(range(_NCORES)))
    total = sum(float(res["partial"][0, 0]) for res in r.results)
    return np.float32(total / _N2)
